# revision 46
# baseline (speedup 1.0000x reference)
"""Trainium2 Bass kernel for nn_LinearDiffusion (truncated Taylor expm(a) @ x).

Math: a = row-normalized symmetric scatter of per-head edge weights onto an
(H, N, N) zero tensor; result = sum_{i=0..6} a^i x / i! with x = h reshaped
per-head.

Strategy (8 NeuronCores, one chip) — all-TensorE sparse SpMM:
  * Gather and scatter are one-hot matmuls on the tensor engine (fp8
    one-hot weights via fast-weight-load, free dim 64 = all 4 heads'
    features concatenated); per-(edge, head) weights applied by the vector
    engine on the PSUM slot tile (fused with the PSUM->SBUF move).
  * STRIP PACKING: cells (dst block I x src block J, ~64 edges each) are
    packed into shared 128-slot PSUM strips (the smallest third of cells
    is split in half by src row range to fill gaps), using max-over-cores
    edge counts so one SPMD program serves all 8 cores; per-core slot
    offsets live purely in the one-hot table CONTENT.  ~344 strips/core
    (vs 512 cells) -> vector-engine weighting work x0.67.
  * Single PSUM output bank accumulates every dst block's scatters: a
    zero-weight "clearing" matmul sets has_written for the whole bank once
    per iteration so all scatters accumulate with start=False; the Taylor
    update then reads PSUM directly (scalar copy for the published x,
    vector STT for the fp32 result).
  * x travels as fp8e4m3 (collectives, SBUF, gather rhs) — halves the
    AllGather transfer, whose tail otherwise dominates the inter-iteration
    critical path; Taylor accumulation stays fp32 (overall rel err ~6e-3).
  * Collectives: slab A = dst blocks I0..4 fires at ~2/3 of the iteration,
    slab B = I5..7 at the end; each window consumes A-class columns before
    B-class so B has ~40%% of an iteration to land.  Outputs double-buffer
    by iteration parity (remote writes must not race the previous
    consumption DMA).  The next iteration's consumption DMA is emitted
    BETWEEN collective A and collective B: consumers are conservatively
    ordered against the collective stream, so emitting it later would make
    it wait for B.  A tiny warmup AllGather absorbs the ncfw first-call
    cost.
"""

import math
from dataclasses import dataclass

import numpy as np

import concourse.bass as bass  # noqa: F401  (kept for callers)
import concourse.tile as tile
from concourse import bacc, mybir
from concourse.bass_utils import run_bass_kernel_spmd

# ----------------------------------------------------------------- config

N, H, E, D = 8192, 4, 131072, 64
d = D // H
NCORES = 8
BLK = 128
NJ = N // BLK            # 64 src blocks
NI = N // NCORES // BLK  # 8 dst blocks per core
NIA = 5                  # dst blocks in collective window A (I0..4)
NJA = NIA * NCORES       # 40 logical column blocks of class A
GRP = 8                  # strips per PSUM tile (one bank)
LOOKAHEAD = 3            # groups the gather runs ahead of the scatter
K_TAYLOR = 6


@dataclass(frozen=True)
class Cfg:
    n: int = N
    n_cores: int = NCORES
    hi_lo_split: bool = True  # accepted for test.py compat; unused

    @property
    def rows_per_core(self):
        return self.n // self.n_cores


# ----------------------------------------------------------- preprocessing


def _entries(e, src, dst, n):
    """Unique symmetric entries with 'last write wins' duplicate semantics,
    matching jax's .at[].set() on CPU. Returns (rows, cols, w[H, nnz])."""
    src = src.astype(np.int64)
    dst = dst.astype(np.int64)
    n_edges = len(src)
    keys = np.concatenate([src * n + dst, dst * n + src])
    eid = np.concatenate([np.arange(n_edges), np.arange(n_edges)])
    order = np.arange(2 * n_edges)
    perm = np.lexsort((-order, keys))
    k_sorted = keys[perm]
    first = np.ones(len(k_sorted), dtype=bool)
    first[1:] = k_sorted[1:] != k_sorted[:-1]
    win = perm[first]
    ukeys = k_sorted[first]
    rows = (ukeys // n).astype(np.int64)
    cols = (ukeys % n).astype(np.int64)
    weids = eid[win]
    vals = e[:, weids].astype(np.float64)  # (H, nnz)
    nheads = e.shape[0]
    rowsum = np.zeros((nheads, n), dtype=np.float64)
    for hh in range(nheads):
        rowsum[hh] = np.bincount(rows, weights=vals[hh], minlength=n)
    w = (vals / rowsum[:, rows]).astype(np.float32)
    return rows, cols, w


# Logical column order: class A = blocks published in collective A (each
# core's dst blocks 0..NIA-1), class B second.
def _logical_j(J):
    k, i = J // NI, J % NI
    return np.where(i < NIA, k * NIA + i, NJA + k * (NI - NIA) + (i - NIA))


_PHYS_OF_LOGICAL = np.zeros(NJ, dtype=np.int64)
_PHYS_OF_LOGICAL[_logical_j(np.arange(NJ))] = np.arange(NJ)


def _build_structure(rows, cols, cfg: Cfg):
    """Shared (core-independent) strip structure from the edge counts.

    Returns per (I, class) a list of strips; each strip is a list of
    gather units (Jlog, rlo, rhi) meaning: edges of cell (I, Jlog) whose
    src row-in-block is in [rlo, rhi) belong to this strip.
    Guarantees max-over-cores total slots per strip <= 128.
    """
    rpc = cfg.rows_per_core
    core = rows // rpc
    I = (rows % rpc) // BLK
    Jl = _logical_j(cols // BLK)
    rin = cols % BLK
    # counts per (core, I, Jlog, half)
    cnt_h = np.zeros((cfg.n_cores, NI, NJ, 2), dtype=np.int64)
    np.add.at(cnt_h, (core, I, Jl, (rin >= 64).astype(int)), 1)

    structure = {}  # (I, cls) -> list of strips (list of units)
    for i in range(NI):
        for cls, Jset in (("A", range(NJA)), ("B", range(NJA, NJ))):
            cells = [(j, cnt_h[:, i, j, :].sum(1)) for j in Jset]
            cells.sort(key=lambda c: -c[1].max())
            nsplit = (len(cells) + 2) // 3
            fulls = cells[:-nsplit]
            halves = []
            for j, _ in cells[-nsplit:]:
                halves.append((j, 0, 64, cnt_h[:, i, j, 0]))
                halves.append((j, 64, 128, cnt_h[:, i, j, 1]))
            halves.sort(key=lambda hv: -hv[3].max())
            strips = []  # (load_vec, [units])
            for j, v in fulls:
                strips.append([v.copy(), [(j, 0, 128)]])
            for j, rlo, rhi, v in halves:
                placed = False
                for s in strips:
                    if (s[0] + v).max() <= 128:
                        s[0] += v
                        s[1].append((j, rlo, rhi))
                        placed = True
                        break
                if not placed:
                    strips.append([v.copy(), [(j, rlo, rhi)]])
            # keep units within a strip ordered by (J, rlo) for locality
            for s in strips:
                s[1].sort()
            structure[(i, cls)] = [s[1] for s in strips]
    return structure


def _structure_key(structure):
    return tuple(
        (i, cls, tuple(tuple(u for u in s) for s in structure[(i, cls)]))
        for i in range(NI)
        for cls in ("A", "B")
    )


def _make_tables(rows, cols, w, structure, cfg: Cfg):
    """Per-core one-hot gather/scatter tables following `structure`.

    Program order of strips: window A (I0..NIA-1) class A, window A class
    B, window B (I NIA..7) class A, window B class B; within each
    (window, class): I-major.
    Returns (strip_order, per-core dict of tables).
    """
    import ml_dtypes

    rpc = cfg.rows_per_core
    core = rows // rpc
    I_all = (rows % rpc) // BLK
    Jl_all = _logical_j(cols // BLK)
    rin_all = cols % BLK

    strip_order = []  # (I, [units])
    for wnd in (range(0, NIA), range(NIA, NI)):
        for cls in ("A", "B"):
            for i in wnd:
                for s in structure[(i, cls)]:
                    strip_order.append((i, s))
    n_strips = len(strip_order)
    n_units = sum(len(s) for _, s in strip_order)

    tables = []
    for k in range(cfg.n_cores):
        m = core == k
        rI = I_all[m]
        rJ = Jl_all[m]
        rr = rin_all[m]
        rdst = (rows[m] % rpc) % BLK
        wv = w[:, m]  # (H, nk)
        # index edges by (I, Jl, rin-half... exact unit match): sort by
        # (I, Jl, rin) then binary-search unit ranges
        order = np.lexsort((rr, rJ, rI))
        rI, rJ, rr, rdst = rI[order], rJ[order], rr[order], rdst[order]
        wv = wv[:, order]
        keys = ((rI * NJ + rJ) * BLK + rr).astype(np.int64)

        goh = np.zeros((128, n_units * 128), dtype=ml_dtypes.float8_e4m3fn)
        sca = np.zeros((128, n_strips * 128), dtype=ml_dtypes.float8_e4m3fn)
        w4 = np.zeros((128, n_strips, H), dtype=np.float32)
        uidx = 0
        for sidx, (i, units) in enumerate(strip_order):
            off = 0
            for j, rlo, rhi in units:
                lo = np.searchsorted(keys, (i * NJ + j) * BLK + rlo)
                hi = np.searchsorted(keys, (i * NJ + j) * BLK + rhi)
                cntu = hi - lo
                if cntu:
                    slots = off + np.arange(cntu)
                    assert off + cntu <= 128, "strip overflow"
                    goh[rr[lo:hi], uidx * 128 + slots] = 1.0
                    sca[slots, sidx * 128 + rdst[lo:hi]] = 1.0
                    w4[slots, sidx, :] = wv[:, lo:hi].T
                    off += cntu
                uidx += 1
            assert off <= 128
        tables.append(
            {
                "goh": goh,
                "sca": sca,
                "w4": np.ascontiguousarray(w4.reshape(128, n_strips * H)),
            }
        )
    return strip_order, tables


# ------------------------------------------------------------ bass program

_FP32 = mybir.dt.float32
_FP16 = mybir.dt.float16
_FP8 = mybir.dt.float8e4


def _build_program(cfg: Cfg, strip_order, phase_lens):
    rpc = cfg.rows_per_core
    n_strips = len(strip_order)
    n_units = sum(len(s) for _, s in strip_order)
    HDA = NIA * D          # collective A payload per partition
    HDB = (NI - NIA) * D   # collective B payload per partition

    nc = bacc.Bacc(
        "TRN2",
        target_bir_lowering=False,
        debug=False,
        num_devices=cfg.n_cores,
    )

    x0p_d = nc.dram_tensor("x0p", [128, NJ * D], _FP8, kind="ExternalInput").ap()
    x0s_d = nc.dram_tensor("x0s", [rpc, D], _FP32, kind="ExternalInput").ap()
    goh_d = nc.dram_tensor(
        "goh", [128, n_units * 128], _FP8, kind="ExternalInput"
    ).ap()
    sca_d = nc.dram_tensor(
        "sca", [128, n_strips * 128], _FP8, kind="ExternalInput"
    ).ap()
    w4_d = nc.dram_tensor("w4", [128, n_strips * H], _FP32, kind="ExternalInput").ap()
    out_d = nc.dram_tensor("out", [rpc, D], _FP32, kind="ExternalOutput").ap()

    slcA = nc.dram_tensor("slcA", [128, HDA], _FP8).ap()
    slcB = nc.dram_tensor("slcB", [128, HDB], _FP8).ap()
    # double-buffered by iteration parity: collective i+1's remote writes
    # must not race the (possibly still draining) consumption DMA of
    # collective i's output
    xgA2 = [
        nc.dram_tensor(f"xgA{p}", [NCORES, 128, HDA], _FP8, addr_space="Shared").ap()
        for p in range(2)
    ]
    xgB2 = [
        nc.dram_tensor(f"xgB{p}", [NCORES, 128, HDB], _FP8, addr_space="Shared").ap()
        for p in range(2)
    ]
    slcW = nc.dram_tensor("slcW", [1, 128], _FP16).ap()
    xgW = nc.dram_tensor("xgW", [NCORES, 128], _FP16, addr_space="Shared").ap()

    groups = [list(range(cfg.n_cores))]

    # group strips into PSUM tiles of GRP strips; groups may span phase
    # boundaries (ordering constraints are per-strip via tile deps)
    strip_groups = []  # list of (start, count)
    s = 0
    while s < n_strips:
        c = min(GRP, n_strips - s)
        strip_groups.append((s, c))
        s += c
    ngrp = len(strip_groups)

    # unit offsets per strip
    unit_off = []
    uo = 0
    for _, units in strip_order:
        unit_off.append(uo)
        uo += len(units)

    # column class of each strip (phases alternate A, B, A, B); per-I
    # scatter accumulation groups must be contiguous per PSUM bank, so
    # class-A strips accumulate into bank A and class-B into bank B
    last_of_I = {}
    for sidx, (i, _) in enumerate(strip_order):
        last_of_I[i] = sidx

    with tile.TileContext(nc) as tc:
        with (
            tc.tile_pool(name="tables", bufs=1) as tp,
            tc.tile_pool(name="xall", bufs=2) as xap,
            tc.tile_pool(name="xgw", bufs=6) as xgp,
            tc.tile_pool(name="acc", bufs=1) as accp,
            tc.tile_pool(name="xnext", bufs=2) as xnp,
            tc.tile_pool(name="psg", bufs=6, space="PSUM") as pgp,
            tc.tile_pool(name="pso", bufs=1, space="PSUM") as pop,
        ):
            goh_t = tp.tile([128, n_units * 128], _FP8, name="goh")
            sca_t = tp.tile([128, n_strips * 128], _FP8, name="sca")
            w4_t = tp.tile([128, n_strips, H], _FP32, name="w4")
            result = accp.tile([128, NI, D], _FP32)

            # warm up the collective firmware while tables stream in
            nc.gpsimd.collective_compute(
                "AllGather",
                mybir.AluOpType.bypass,
                replica_groups=groups,
                ins=[slcW],
                outs=[xgW],
            )

            # table loads in consumption order: goh chunks follow the unit
            # order; interleave sca/w4 by phase so compute streams behind
            xallA = xap.tile([128, NJA, D], _FP8, tag="xa")
            xallB = xap.tile([128, NJ - NJA, D], _FP8, tag="xb")
            nc.sync.dma_start(
                out=xallA[:],
                in_=x0p_d[:, 0 : NJA * D].rearrange("p (j f) -> p j f", f=D),
            )
            nc.sync.dma_start(
                out=xallB[:],
                in_=x0p_d[:, NJA * D :].rearrange("p (j f) -> p j f", f=D),
            )
            nc.sync.dma_start(
                out=result[:],
                in_=x0s_d.rearrange("(j p) f -> p j f", p=128),
            )
            # chunked table DMAs interleaved in program-consumption order so
            # iteration-1 scatters are not starved behind all gather tables
            NCHUNK = 16
            ulo = slo = 0
            for ci in range(NCHUNK):
                uhi = (ci + 1) * n_units // NCHUNK
                shi = (ci + 1) * n_strips // NCHUNK
                if uhi > ulo:
                    nc.sync.dma_start(
                        out=goh_t[:, ulo * 128 : uhi * 128],
                        in_=goh_d[:, ulo * 128 : uhi * 128],
                    )
                if shi > slo:
                    nc.sync.dma_start(
                        out=sca_t[:, slo * 128 : shi * 128],
                        in_=sca_d[:, slo * 128 : shi * 128],
                    )
                    nc.sync.dma_start(
                        out=w4_t[:, slo:shi, :].rearrange("p c h -> p (c h)"),
                        in_=w4_d[:, slo * H : shi * H],
                    )
                ulo, slo = uhi, shi

            out_ps = pop.tile([128, NI, D], _FP32, name="ops")
            zero_t = tp.tile([128, 128], _FP8, name="zt")
            nc.gpsimd.memset(zero_t[:], 0)

            for it in range(1, K_TAYLOR + 1):
                coef = 1.0 / math.factorial(it)
                xnext = xnp.tile([128, NI, D], _FP8, tag="xnext")
                state = {}

                # dummy zero matmul: writes 0 to the whole out bank with
                # start=True, setting has_written everywhere so all of this
                # iteration's scatters can accumulate with start=False (a
                # per-I start=True would clear the whole bank's bits and
                # break other blocks' in-flight accumulations)
                nc.tensor.matmul(
                    out_ps[:].rearrange("p i f -> p (i f)"),
                    lhsT=zero_t[:],
                    rhs=sca_t[:, 0:512],
                    start=True,
                    stop=True,
                )

                pend = [None] * ngrp

                def emit_gather(t, xallA, xallB):
                    g0, gc = strip_groups[t]
                    ps_g = pgp.tile([128, GRP, D], _FP32, tag="psg")
                    for si in range(g0, g0 + gc):
                        _, units = strip_order[si]
                        for uu, (j, rlo, rhi) in enumerate(units):
                            uidx = unit_off[si] + uu
                            xsrc = (
                                xallA[:, j, :]
                                if j < NJA
                                else xallB[:, j - NJA, :]
                            )
                            nc.tensor.matmul(
                                ps_g[:, si - g0, :],
                                lhsT=goh_t[:, uidx * 128 : uidx * 128 + 128],
                                rhs=xsrc,
                                start=(uu == 0),
                                stop=(uu == len(units) - 1),
                            )
                    xgw_g = xgp.tile([128, GRP, D], _FP16, tag="xgw")
                    wvb = (
                        w4_t[:, g0 : g0 + gc, :]
                        .unsqueeze(3)
                        .to_broadcast([128, gc, H, d])
                    )
                    nc.vector.tensor_mul(
                        xgw_g[:, 0:gc, :].rearrange("p j (h f) -> p j h f", h=H),
                        ps_g[:, 0:gc, :].rearrange("p j (h f) -> p j h f", h=H),
                        wvb,
                    )
                    pend[t] = xgw_g

                def emit_scatter(s, it, coef, xnext, state):
                    g0, gc = strip_groups[s]
                    xgw_s = pend[s]
                    pend[s] = None
                    for si in range(g0, g0 + gc):
                        i_blk, _ = strip_order[si]
                        nc.tensor.matmul(
                            out_ps[:, i_blk, :],
                            lhsT=sca_t[:, si * 128 : si * 128 + 128],
                            rhs=xgw_s[:, si - g0, :],
                            start=False,
                            stop=(si == last_of_I[i_blk]),
                        )
                        if si != last_of_I[i_blk]:
                            continue
                        # dst block complete: Taylor step straight from PSUM
                        nc.scalar.copy(xnext[:, i_blk, :], out_ps[:, i_blk, :])
                        nc.vector.scalar_tensor_tensor(
                            result[:, i_blk, :],
                            out_ps[:, i_blk, :],
                            coef,
                            result[:, i_blk, :],
                            op0=mybir.AluOpType.mult,
                            op1=mybir.AluOpType.add,
                        )
                        if it == K_TAYLOR:
                            nc.sync.dma_start(
                                out=out_d[i_blk * 128 : (i_blk + 1) * 128, :],
                                in_=result[:, i_blk, :],
                            )
                            continue
                        xgA = xgA2[it % 2]
                        xgB = xgB2[it % 2]
                        if i_blk == NIA - 1:
                            nc.sync.dma_start(
                                out=slcA,
                                in_=xnext[:, 0:NIA, :].rearrange(
                                    "p i f -> p (i f)"
                                ),
                            )
                            nc.gpsimd.collective_compute(
                                "AllGather",
                                mybir.AluOpType.bypass,
                                replica_groups=groups,
                                ins=[slcA],
                                outs=[xgA],
                            )
                            # consumption DMA emitted between this collective
                            # and the kicker: its conservative stream wait
                            # then points at THIS collective...
                            xa = xap.tile([128, NJA, D], _FP8, tag="xa")
                            nc.sync.dma_start(
                                out=xa[:].rearrange(
                                    "p (k i) f -> p k i f", k=NCORES
                                ),
                                in_=xgA.rearrange("k p (i f) -> p k i f", f=D),
                            )
                            state["xallA_next"] = xa
                        elif i_blk == NI - 1:
                            nc.sync.dma_start(
                                out=slcB,
                                in_=xnext[:, NIA:NI, :].rearrange(
                                    "p i f -> p (i f)"
                                ),
                            )
                            nc.gpsimd.collective_compute(
                                "AllGather",
                                mybir.AluOpType.bypass,
                                replica_groups=groups,
                                ins=[slcB],
                                outs=[xgB],
                            )
                            xb = xap.tile([128, NJ - NJA, D], _FP8, tag="xb")
                            nc.sync.dma_start(
                                out=xb[:].rearrange(
                                    "p (k i) f -> p k i f", k=NCORES
                                ),
                                in_=xgB.rearrange("k p (i f) -> p k i f", f=D),
                            )
                            state["xallB_next"] = xb

                for t in range(ngrp + LOOKAHEAD):
                    s = t - LOOKAHEAD
                    if s >= 0:
                        emit_scatter(s, it, coef, xnext, state)
                    if t < ngrp:
                        emit_gather(t, xallA, xallB)
                if it < K_TAYLOR:
                    xallA = state["xallA_next"]
                    xallB = state["xallB_next"]

    nc.compile()
    return nc


# ------------------------------------------------------------------ driver

_CACHE = {}


def _get_program(cfg: Cfg, strip_order, phase_lens, skey):
    key = (cfg, skey)
    if key not in _CACHE:
        _CACHE[key] = _build_program(cfg, strip_order, phase_lens)
    return _CACHE[key]


def run(h, e, src, dst, cfg: Cfg = Cfg(), trace: bool = False):
    """Full pipeline: preprocess, build/compile (cached), execute, assemble."""
    h = np.asarray(h, dtype=np.float32)
    e = np.asarray(e, dtype=np.float32)
    src = np.asarray(src)
    dst = np.asarray(dst)
    nheads = e.shape[0]
    n = h.shape[0]
    dd = h.shape[1] // nheads
    assert (n, nheads, dd) == (cfg.n, H, d), (n, nheads, dd)

    rows, cols, w = _entries(e, src, dst, n)
    structure = _build_structure(rows, cols, cfg)
    strip_order, tables = _make_tables(rows, cols, w, structure, cfg)
    # phase lengths (strip counts) in program order
    phase_lens = []
    for wnd in (range(0, NIA), range(NIA, NI)):
        for cls in ("A", "B"):
            phase_lens.append(
                sum(len(structure[(i, cls)]) for i in wnd)
            )
    skey = _structure_key(structure)

    x0 = np.ascontiguousarray(
        h.reshape(nheads, n, dd).transpose(1, 0, 2).reshape(n, nheads * dd)
    )
    # (128, NJ, D) fp16 in logical column order, partition-major
    import ml_dtypes

    x0p = np.ascontiguousarray(
        x0.astype(ml_dtypes.float8_e4m3fn)
        .reshape(NJ, 128, D)[_PHYS_OF_LOGICAL]
        .transpose(1, 0, 2)
    ).reshape(128, NJ * D)
    rpc = cfg.rows_per_core
    in_maps = [
        {
            "x0p": x0p,
            "x0s": np.ascontiguousarray(x0[k * rpc : (k + 1) * rpc]),
            "goh": t["goh"],
            "sca": t["sca"],
            "w4": t["w4"],
        }
        for k, t in enumerate(tables)
    ]
    nc = _get_program(cfg, strip_order, phase_lens, skey)
    res = run_bass_kernel_spmd(
        nc,
        in_maps,
        list(range(cfg.n_cores)),
        trace=trace,
    )
    out = np.concatenate(
        [res.results[k]["out"] for k in range(cfg.n_cores)], axis=0
    )
    out = np.ascontiguousarray(
        out.reshape(n, nheads, dd).transpose(1, 0, 2)
    ).reshape(n, nheads * dd)
    return out, res


def kernel(h, e, src, dst):
    out, _ = run(h, e, src, dst)
    return out
